# revision 15
# baseline (speedup 1.0000x reference)
"""Trainium2 Bass kernel for nn_CosineSimilarity (prototype cosine-sim + MLP).

Data-parallel over the query dimension across 8 NeuronCores; the support
set, prototypes and MLP weights are replicated on every core.

Self-contained: hardcodes all shapes. kernel(**inputs) -> (similarity, cos).
"""

import sys

sys.path.append("/opt/trn_rl_repo")

import numpy as np
import ml_dtypes

import concourse.bacc as bacc
import concourse.mybir as mybir
from concourse import tile
from concourse.bass_utils import run_bass_kernel_spmd

# Problem shapes
NC = 16  # classes
C = 128  # channels
NS = 8192  # support points
NQ = 30000  # query points
EPS = 1e-8

N_CORES = 8
QSHARD = NQ // N_CORES  # 3750
QPAD = 3840  # 30 chunks of 128
N_CHUNK = QPAD // 128  # 30
SUP_CHUNKS = NS // 128  # 64

F32 = mybir.dt.float32
F32R = mybir.dt.float32r
BF16 = mybir.dt.bfloat16

# Final matmul dtype knob: BF16 (fast) or F32 (exact, 4x slower on PE)
FINAL_MM_BF16 = True

_PROG_CACHE = {}


def _build_program():
    nc = bacc.Bacc(num_devices=N_CORES)

    # ---- I/O ----
    q_d = nc.dram_tensor("q", [QPAD, C], F32, kind="ExternalInput").ap()
    sup_d = nc.dram_tensor("sup", [NS, C], F32, kind="ExternalInput").ap()
    lab_d = nc.dram_tensor("lab", [SUP_CHUNKS, 128], F32, kind="ExternalInput").ap()
    wq_d = nc.dram_tensor("wq", [C, C], F32R, kind="ExternalInput").ap()
    wp_d = nc.dram_tensor("wp", [C, C], F32, kind="ExternalInput").ap()
    b1_d = nc.dram_tensor("b1", [C, 1], F32, kind="ExternalInput").ap()
    w2_dt = BF16 if FINAL_MM_BF16 else F32
    w2_d = nc.dram_tensor("w2", [C, C], w2_dt, kind="ExternalInput").ap()
    b2r_d = nc.dram_tensor("b2r", [128, 4 * C], F32, kind="ExternalInput").ap()
    id_d = nc.dram_tensor("ident", [128, 128], F32, kind="ExternalInput").ap()
    iota_d = nc.dram_tensor("iota16", [128, NC], F32, kind="ExternalInput").ap()
    onec_d = nc.dram_tensor("onec", [128, 1], F32, kind="ExternalInput").ap()
    oner_d = nc.dram_tensor("oner", [1, 128], F32, kind="ExternalInput").ap()
    # per-class masked Wc: wcm[:, n*128:(n+1)*128][m, j] = Wc[j] * (m == n)
    wcm_d = nc.dram_tensor("wcm", [NC, NC * C], F32R, kind="ExternalInput").ap()

    sim_d = nc.dram_tensor("sim", [QPAD, NC, C], F32, kind="ExternalOutput").ap()
    cos_d = nc.dram_tensor("cosv", [QPAD, NC], F32, kind="ExternalOutput").ap()

    with tile.TileContext(nc) as tc:
        with (
            tc.tile_pool(name="wgt", bufs=1) as wgt,
            tc.tile_pool(name="sup", bufs=3) as supp,
            tc.tile_pool(name="oh", bufs=6) as ohp,
            tc.tile_pool(name="qin", bufs=2) as qinp,
            tc.tile_pool(name="small", bufs=4) as smp,
            tc.tile_pool(name="qt", bufs=2) as qtp,
            tc.tile_pool(name="ct", bufs=2) as ctp,
            tc.tile_pool(name="h", bufs=6) as hp,
            tc.tile_pool(name="ob", bufs=2) as obp,
            tc.tile_pool(name="php", bufs=1, space="PSUM") as php,
            tc.tile_pool(name="psp", bufs=1, space="PSUM") as psp,
            tc.tile_pool(name="pps", bufs=1, space="PSUM") as pps,
            tc.tile_pool(name="pacc", bufs=1, space="PSUM") as pacc,
        ):
            # ---- load weights / constants ----
            def _load(dram, shape, dt=F32, tag=None):
                t = wgt.tile(shape, dt, tag=tag)
                nc.sync.dma_start(t[:], dram)
                return t

            wq_s = _load(wq_d, [C, C], dt=F32R, tag="wq")
            wp_s = _load(wp_d, [C, C], tag="wp")
            b1_s = _load(b1_d, [C, 1], tag="b1")
            w2_s = _load(w2_d, [C, C], w2_dt, tag="w2")
            b2r4_s = _load(b2r_d, [128, 4 * C], tag="b2r")
            id_s = _load(id_d, [128, 128], tag="ident")
            iota_s = _load(iota_d, [128, NC], tag="iota")
            onec_s = _load(onec_d, [128, 1], tag="onec")
            oner_s = _load(oner_d, [1, 128], tag="oner")
            wcm_s = _load(wcm_d, [NC, NC * C], dt=F32R, tag="wcm")
            lab_s = _load(lab_d, [SUP_CHUNKS, 128], tag="lab")

            # ---- prototypes: segment sum via onehot matmuls ----
            labT_p = pps.tile([128, SUP_CHUNKS], F32, tag="pp")
            nc.tensor.transpose(labT_p[:], lab_s[:], id_s[:SUP_CHUNKS, :SUP_CHUNKS])
            labT_s = wgt.tile([128, SUP_CHUNKS], F32, tag="labT")
            nc.vector.tensor_copy(labT_s[:], labT_p[:])

            acc_p = pacc.tile([C, NC], F32, tag="pp_sums")
            sums_p = acc_p[:, 0:NC]
            oha_s = wgt.tile([128, NC], F32, tag="oha")
            nc.vector.memset(oha_s[:], 0.0)
            for kb in range(SUP_CHUNKS // 4):
                sb = supp.tile([128, 4 * C], F32)
                nc.sync.dma_start(
                    sb[:],
                    sup_d[kb * 512 : (kb + 1) * 512, :].rearrange(
                        "(a p) c -> p a c", p=128
                    ),
                )
                for a in range(4):
                    k = kb * 4 + a
                    sk = sb[:, a * C : (a + 1) * C]
                    oh = ohp.tile([128, NC], F32)
                    nc.vector.tensor_scalar(
                        oh[:], iota_s[:], labT_s[:, k : k + 1], None,
                        mybir.AluOpType.is_equal,
                    )
                    nc.tensor.matmul(
                        sums_p, sk, oh[:],
                        start=(k == 0), stop=(k == SUP_CHUNKS - 1),
                    )
                    nc.vector.tensor_tensor(
                        oha_s[:], oha_s[:], oh[:], mybir.AluOpType.add
                    )

            # protosT = sumsT * (1/max(cnt,1)) ; pnT = protosT / max(||proto||,eps)
            cnt_p = pps.tile([1, NC], F32, tag="pp")
            nc.tensor.matmul(cnt_p[:], onec_s[:], oha_s[:], start=True, stop=True)
            cnt_s = smp.tile([1, NC], F32, tag="sm1")
            nc.vector.tensor_copy(cnt_s[:], cnt_p[:])
            nc.vector.tensor_scalar_max(cnt_s[:], cnt_s[:], 1.0)
            icnt_s = smp.tile([1, NC], F32, tag="sm2")
            nc.vector.reciprocal(icnt_s[:], cnt_s[:])
            icr_p = pps.tile([128, NC], F32, tag="pp")
            nc.tensor.matmul(icr_p[:], oner_s[:], icnt_s[:], start=True, stop=True)
            icr_s = smp.tile([128, NC], F32, tag="sm3")
            nc.vector.tensor_copy(icr_s[:], icr_p[:])
            protT_s = wgt.tile([C, NC], F32, tag="protT")
            nc.vector.tensor_tensor(
                protT_s[:], sums_p, icr_s[:], mybir.AluOpType.mult
            )
            # proto norms (reduce over c = partitions, via ones matmul)
            sqp_s = smp.tile([C, NC], F32, tag="sm3")
            nc.vector.tensor_tensor(sqp_s[:], protT_s[:], protT_s[:], mybir.AluOpType.mult)
            nrm_p = pps.tile([1, NC], F32, tag="pp")
            nc.tensor.matmul(nrm_p[:], onec_s[:], sqp_s[:], start=True, stop=True)
            nrm_s = smp.tile([1, NC], F32, tag="sm1")
            nc.scalar.activation(nrm_s[:], nrm_p[:], mybir.ActivationFunctionType.Sqrt)
            nc.vector.tensor_scalar_max(nrm_s[:], nrm_s[:], EPS)
            ipn_s = smp.tile([1, NC], F32, tag="sm2")
            nc.vector.reciprocal(ipn_s[:], nrm_s[:])
            ipr_p = pps.tile([128, NC], F32, tag="pp")
            nc.tensor.matmul(ipr_p[:], oner_s[:], ipn_s[:], start=True, stop=True)
            ipr_s = smp.tile([128, NC], F32, tag="sm3")
            nc.vector.tensor_copy(ipr_s[:], ipr_p[:])
            pnT_s = wgt.tile([C, NC], F32R, tag="pnT")
            nc.vector.tensor_tensor(pnT_s[:], protT_s[:], ipr_s[:], mybir.AluOpType.mult)
            # pWb = Wp.T @ protosT + b1  [h, NC]
            pwb_p = pps.tile([C, NC], F32, tag="pp")
            nc.tensor.matmul(pwb_p[:], wp_s[:], protT_s[:], start=True, stop=True)
            pwb_s = wgt.tile([C, NC], F32, tag="pwb")
            nc.vector.tensor_scalar(
                pwb_s[:], pwb_p[:], b1_s[:, 0:1], None, mybir.AluOpType.add
            )

            # ---- main: per query group of up to 512 ----
            groups = []
            base = 0
            while base < N_CHUNK:
                w = min(4, N_CHUNK - base)
                groups.append((base, w))
                base += w

            for gbase, gw in groups:
                W = gw * 128
                qT = qtp.tile([C, 512], F32R, tag="qT")
                cosT = ctp.tile([NC, 512], F32R, tag="cosT")
                # one batched query load for the whole group
                qb = qinp.tile([128, 4 * C], F32, tag="qb")
                nc.sync.dma_start(
                    qb[:, :W],
                    q_d[gbase * 128 : gbase * 128 + W, :].rearrange(
                        "(a p) c -> p a c", p=128
                    ),
                )
                for j in range(gw):
                    i = gbase + j
                    Q = qb[:, j * C : (j + 1) * C]
                    # 1/max(||q||,eps) per row
                    sq = smp.tile([128, C], F32, tag="sq")
                    nc.vector.tensor_tensor(sq[:], Q, Q, mybir.AluOpType.mult)
                    s2 = smp.tile([128, 1], F32, tag="s2")
                    nc.vector.reduce_sum(s2[:], sq[:], axis=mybir.AxisListType.X)
                    nrmq = smp.tile([128, 1], F32, tag="nq")
                    nc.scalar.activation(
                        nrmq[:], s2[:], mybir.ActivationFunctionType.Sqrt
                    )
                    nc.vector.tensor_scalar_max(nrmq[:], nrmq[:], EPS)
                    invq = smp.tile([128, 1], F32, tag="iq")
                    nc.vector.reciprocal(invq[:], nrmq[:])
                    # qT chunk
                    qt_p = pps.tile([C, 128], F32, tag="pp")
                    nc.tensor.transpose(qt_p[:], Q, id_s[:])
                    nc.vector.tensor_copy(qT[:, j * 128 : (j + 1) * 128], qt_p[:])
                    # cos chunk: (q @ pnT) * invq
                    sqn_p = pps.tile([128, NC], F32, tag="pp")
                    nc.tensor.matmul(
                        sqn_p[:], qT[:, j * 128 : (j + 1) * 128], pnT_s[:],
                        start=True, stop=True,
                    )
                    coss = smp.tile([128, NC], F32, tag="coss")
                    nc.scalar.activation(
                        coss[:], sqn_p[:], mybir.ActivationFunctionType.Copy,
                        scale=invq[:, 0:1],
                    )
                    nc.sync.dma_start(cos_d[i * 128 : (i + 1) * 128, :], coss[:])
                    # cosT chunk
                    ct_p = pps.tile([NC, 128], F32, tag="pp")
                    nc.tensor.transpose(ct_p[:], coss[:], id_s[:])
                    nc.vector.tensor_copy(cosT[:, j * 128 : (j + 1) * 128], ct_p[:])

                obs = [obp.tile([128, NC * C], F32, tag=f"ob{j}", name=f"ob{j}") for j in range(gw)]
                for ng in range(NC // 4):
                    hs = []
                    phs = []
                    # paired mm1 so Wq stays loaded across consecutive matmuls
                    for dn in range(4):
                        ph = php.tile([C, 512], F32, tag=f"ph{dn % 2}")
                        nc.tensor.matmul(
                            ph[:, :W], wq_s[:], qT[:, :W],
                            start=True, stop=False,
                        )
                        phs.append(ph)
                        if dn % 2 == 1:
                            for b in (0, 1):
                                n = ng * 4 + dn - 1 + b
                                nc.tensor.matmul(
                                    phs[dn - 1 + b][:, :W],
                                    wcm_s[:, n * C : (n + 1) * C],
                                    cosT[:, :W],
                                    start=False, stop=True,
                                )
                            for b in (0, 1):
                                n = ng * 4 + dn - 1 + b
                                h = hp.tile([C, 512], w2_dt, tag="h")
                                nc.scalar.activation(
                                    h[:, :W], phs[dn - 1 + b][:, :W],
                                    mybir.ActivationFunctionType.Relu,
                                    bias=pwb_s[:, n : n + 1],
                                )
                                hs.append(h)
                    for j in range(gw):
                        pg = psp.tile([128, 4 * C], F32, tag=f"psg{j}")
                        for dn in range(4):
                            nc.tensor.matmul(
                                pg[:, dn * C : (dn + 1) * C],
                                hs[dn][:, j * 128 : (j + 1) * 128], w2_s[:],
                                start=True, stop=True,
                            )
                        nc.vector.tensor_tensor(
                            obs[j][:, ng * 4 * C : (ng + 1) * 4 * C],
                            pg[:], b2r4_s[:], mybir.AluOpType.add,
                        )
                for j in range(gw):
                    q0 = (gbase + j) * 128
                    nc.sync.dma_start(
                        sim_d[q0 : q0 + 128, :, :].rearrange("p n c -> p (n c)"),
                        obs[j][:],
                    )

    nc.finalize()
    return nc


def _prep_inputs(support_features, support_labels, query_features, W1, b1, W2, b2):
    """Host-side shard + constant prep. Returns per-core input maps."""
    sup = np.ascontiguousarray(np.asarray(support_features, np.float32))
    lab = np.asarray(support_labels).astype(np.float32).reshape(SUP_CHUNKS, 128)
    qf = np.ascontiguousarray(np.asarray(query_features, np.float32))
    W1 = np.asarray(W1, np.float32)
    b1 = np.asarray(b1, np.float32)
    W2 = np.asarray(W2, np.float32)
    b2 = np.asarray(b2, np.float32)

    wq = np.ascontiguousarray(W1[:C])
    wp = np.ascontiguousarray(W1[C : 2 * C])
    wc = W1[2 * C]
    wcm = np.zeros((NC, NC * C), np.float32)
    for n in range(NC):
        wcm[n, n * C : (n + 1) * C] = wc
    b1c = np.ascontiguousarray(b1.reshape(C, 1))
    w2 = np.ascontiguousarray(W2.astype(ml_dtypes.bfloat16 if FINAL_MM_BF16 else np.float32))
    b2r = np.ascontiguousarray(np.tile(b2.reshape(1, C), (128, 4)))
    ident = np.eye(128, dtype=np.float32)
    iota16 = np.tile(np.arange(NC, dtype=np.float32), (128, 1))
    onec = np.ones((128, 1), np.float32)
    oner = np.ones((1, 128), np.float32)

    in_maps = []
    for i in range(N_CORES):
        shard = np.zeros((QPAD, C), np.float32)
        shard[:QSHARD] = qf[i * QSHARD : (i + 1) * QSHARD]
        in_maps.append(
            {
                "q": shard, "sup": sup, "lab": lab, "wq": wq, "wp": wp,
                "wcm": wcm, "b1": b1c, "w2": w2, "b2r": b2r, "ident": ident,
                "iota16": iota16, "onec": onec, "oner": oner,
            }
        )
    return in_maps


def _run(inputs, trace=False):
    if "prog" not in _PROG_CACHE:
        _PROG_CACHE["prog"] = _build_program()
    nc = _PROG_CACHE["prog"]
    in_maps = _prep_inputs(**inputs)
    res = run_bass_kernel_spmd(nc, in_maps, list(range(N_CORES)), trace=trace)
    sim = np.concatenate([res.results[i]["sim"][:QSHARD] for i in range(N_CORES)], 0)
    cos = np.concatenate([res.results[i]["cosv"][:QSHARD] for i in range(N_CORES)], 0)
    return (sim, cos[:, :, None].astype(np.float32)), res


def kernel(**inputs):
    out, _ = _run(inputs, trace=False)
    return out


# revision 50
# speedup vs baseline: 1.0983x; 1.0983x over previous
"""Trainium2 Bass kernel for nn_CosineSimilarity (prototype cosine-sim + MLP).

Data-parallel over the query dimension across 8 NeuronCores; the support
set, prototypes and MLP weights are replicated on every core.

Self-contained: hardcodes all shapes. kernel(**inputs) -> (similarity, cos).
"""

import sys

sys.path.append("/opt/trn_rl_repo")

import numpy as np
import ml_dtypes

import concourse.bacc as bacc
import concourse.mybir as mybir
from concourse import tile
from concourse.bass_utils import run_bass_kernel_spmd

# Problem shapes
NC = 16  # classes
C = 128  # channels
NS = 8192  # support points
NQ = 30000  # query points
EPS = 1e-8

N_CORES = 8
QSHARD = NQ // N_CORES  # 3750
QPAD = 3840  # 30 chunks of 128
N_CHUNK = QPAD // 128  # 30
SUP_CHUNKS = NS // 128  # 64

F32 = mybir.dt.float32
F32R = mybir.dt.float32r
BF16 = mybir.dt.bfloat16

# Final matmul dtype knob: BF16 (fast) or F32 (exact, 4x slower on PE)
FINAL_MM_BF16 = True

_PROG_CACHE = {}


def _build_program():
    nc = bacc.Bacc(num_devices=N_CORES)

    # ---- I/O ----
    q_d = nc.dram_tensor("q", [QPAD, C], F32, kind="ExternalInput").ap()
    sup_d = nc.dram_tensor("sup", [NS, C], F32, kind="ExternalInput").ap()
    lab_d = nc.dram_tensor("lab", [SUP_CHUNKS, 128], F32, kind="ExternalInput").ap()
    wq_d = nc.dram_tensor("wq", [C, C], F32R, kind="ExternalInput").ap()
    wp_d = nc.dram_tensor("wp", [C, C], F32, kind="ExternalInput").ap()
    b1_d = nc.dram_tensor("b1", [C, 1], F32, kind="ExternalInput").ap()
    w2_dt = BF16 if FINAL_MM_BF16 else F32
    w2_d = nc.dram_tensor("w2", [C, C], w2_dt, kind="ExternalInput").ap()
    b2r_d = nc.dram_tensor("b2r", [128, 4 * C], F32, kind="ExternalInput").ap()
    id_d = nc.dram_tensor("ident", [128, 128], F32, kind="ExternalInput").ap()
    iota_d = nc.dram_tensor("iota16", [128, NC], F32, kind="ExternalInput").ap()
    onec_d = nc.dram_tensor("onec", [128, 1], F32, kind="ExternalInput").ap()
    oner_d = nc.dram_tensor("oner", [1, 128], F32, kind="ExternalInput").ap()
    # per-class masked Wc: wcm[:, n*128:(n+1)*128][m, j] = Wc[j] * (m == n)
    wcm_d = nc.dram_tensor("wcm", [NC, NC * C], F32R, kind="ExternalInput").ap()

    sim_d = nc.dram_tensor("sim", [QPAD, NC, C], F32, kind="ExternalOutput").ap()
    cos_d = nc.dram_tensor("cosv", [QPAD, NC], F32, kind="ExternalOutput").ap()

    with tile.TileContext(nc) as tc:
        with (
            tc.tile_pool(name="wgt", bufs=1) as wgt,
            tc.tile_pool(name="sup", bufs=1) as supp,
            tc.tile_pool(name="oh", bufs=6) as ohp,
            tc.tile_pool(name="qin", bufs=2) as qinp,
            tc.tile_pool(name="small", bufs=4) as smp,
            tc.tile_pool(name="qt", bufs=2) as qtp,
            tc.tile_pool(name="ct", bufs=2) as ctp,
            tc.tile_pool(name="h", bufs=8) as hp,
            tc.tile_pool(name="ob", bufs=3) as obp,
            tc.tile_pool(name="php", bufs=1, space="PSUM") as php,
            tc.tile_pool(name="psp", bufs=1, space="PSUM") as psp,
            tc.tile_pool(name="pps", bufs=2, space="PSUM") as pps,
            tc.tile_pool(name="pacc", bufs=1, space="PSUM") as pacc,
        ):
            # ---- load weights / constants ----
            def _load(dram, shape, dt=F32, tag=None):
                t = wgt.tile(shape, dt, tag=tag)
                nc.sync.dma_start(t[:], dram)
                return t

            # support + labels first: the prototype chain is the critical path
            lab_s = _load(lab_d, [SUP_CHUNKS, 128], tag="lab")
            id_s = _load(id_d, [128, 128], tag="ident")
            iota_s = _load(iota_d, [128, NC], tag="iota")
            qall = wgt.tile([128, N_CHUNK * C], F32, tag="qall")
            nc.sync.dma_start(
                qall[:, : 8 * C].rearrange("p (a c) -> p a c", c=C),
                q_d[0:1024, :].rearrange("(a p) c -> p a c", p=128),
            )
            sup_tiles = []
            for sb16 in range(4):
                st = supp.tile([128, 16 * C], F32, name=f"sup{sb16}")
                nc.sync.dma_start(
                    st[:],
                    sup_d[sb16 * 2048 : (sb16 + 1) * 2048, :].rearrange(
                        "(a p) c -> p a c", p=128
                    ),
                )
                sup_tiles.append(st)
            for h2 in range(2):
                lo = 1024 if h2 == 0 else 1920
                nc.sync.dma_start(
                    qall[:, lo // 128 * C : (h2 + 1) * 15 * C].rearrange(
                        "p (a c) -> p a c", c=C
                    ),
                    q_d[lo : (h2 + 1) * 1920, :].rearrange(
                        "(a p) c -> p a c", p=128
                    ),
                )
            onec_s = _load(onec_d, [128, 1], tag="onec")
            oner_s = _load(oner_d, [1, 128], tag="oner")
            wq_s = _load(wq_d, [C, C], dt=F32R, tag="wq")
            wp_s = _load(wp_d, [C, C], tag="wp")
            b1_s = _load(b1_d, [C, 1], tag="b1")
            w2_s = _load(w2_d, [C, C], w2_dt, tag="w2")
            b2r4_s = _load(b2r_d, [128, 4 * C], tag="b2r")
            wcm_s = _load(wcm_d, [NC, NC * C], dt=F32R, tag="wcm")

            # ---- prototypes: segment sum via onehot matmuls ----
            labT_p = pps.tile([128, SUP_CHUNKS], F32, tag="pp")
            nc.tensor.transpose(labT_p[:], lab_s[:], id_s[:SUP_CHUNKS, :SUP_CHUNKS])
            labT_s = wgt.tile([128, SUP_CHUNKS], F32, tag="labT")
            nc.vector.tensor_copy(labT_s[:], labT_p[:])

            acc_p = pacc.tile([C, NC], F32, tag="pp_sums")
            sums_p = acc_p[:, 0:NC]
            oha_s = wgt.tile([128, NC], F32, tag="oha")
            nc.vector.memset(oha_s[:], 0.0)
            for kb in range(4):
                sb = sup_tiles[kb]
                for a in range(16):
                    k = kb * 16 + a
                    sk = sb[:, a * C : (a + 1) * C]
                    oh = ohp.tile([128, NC], F32)
                    nc.vector.tensor_scalar(
                        oh[:], iota_s[:], labT_s[:, k : k + 1], None,
                        mybir.AluOpType.is_equal,
                    )
                    nc.tensor.matmul(
                        sums_p, sk, oh[:],
                        start=(k == 0), stop=(k == SUP_CHUNKS - 1),
                    )
                    nc.vector.tensor_tensor(
                        oha_s[:], oha_s[:], oh[:], mybir.AluOpType.add
                    )

            # critical path: unnormalized sumsT straight to SBUF for the cos matmuls
            sumsT_sb = wgt.tile([C, NC], F32R, tag="sumsT")
            nc.vector.tensor_copy(sumsT_sb[:], sums_p)
            # per-class 1/||sums|| (parallel chain; consumed by cosT scaling)
            sqp_s = smp.tile([C, NC], F32, tag="sm3")
            nc.vector.tensor_tensor(sqp_s[:], sumsT_sb[:].bitcast(F32), sumsT_sb[:].bitcast(F32), mybir.AluOpType.mult)
            nrm_p = pps.tile([1, NC], F32, tag="pp")
            nc.tensor.matmul(nrm_p[:], onec_s[:], sqp_s[:], start=True, stop=True)
            nrm_s = smp.tile([1, NC], F32, tag="sm1")
            nc.scalar.activation(nrm_s[:], nrm_p[:], mybir.ActivationFunctionType.Sqrt)
            ipn_s = wgt.tile([1, NC], F32, tag="ipn")
            nc.vector.reciprocal(ipn_s[:], nrm_s[:])
            # ipn as [NC,1] column for per-partition scaling of cosT
            ipnc_p = pps.tile([NC, 1], F32, tag="pp")
            nc.tensor.transpose(ipnc_p[:], ipn_s[:], id_s[:1, :1])
            ipnc_s = wgt.tile([NC, 1], F32, tag="ipnc")
            nc.vector.tensor_copy(ipnc_s[:], ipnc_p[:])
            # ipn replicated [128, NC] for the deferred cos-output fixup
            ipr_p = pps.tile([128, NC], F32, tag="pp")
            nc.tensor.matmul(ipr_p[:], oner_s[:], ipn_s[:], start=True, stop=True)
            ipr_s = wgt.tile([128, NC], F32, tag="iprr")
            nc.vector.tensor_copy(ipr_s[:], ipr_p[:])

            # protos + pWb (needed only at first relu — off critical path)
            cnt_p = pps.tile([1, NC], F32, tag="pp")
            nc.tensor.matmul(cnt_p[:], onec_s[:], oha_s[:], start=True, stop=True)
            cnt_s = smp.tile([1, NC], F32, tag="sm1")
            nc.vector.tensor_copy(cnt_s[:], cnt_p[:])
            nc.vector.tensor_scalar_max(cnt_s[:], cnt_s[:], 1.0)
            icnt_s = smp.tile([1, NC], F32, tag="sm2")
            nc.vector.reciprocal(icnt_s[:], cnt_s[:])
            icr_p = pps.tile([128, NC], F32, tag="pp")
            nc.tensor.matmul(icr_p[:], oner_s[:], icnt_s[:], start=True, stop=True)
            icr_s = smp.tile([128, NC], F32, tag="sm3")
            nc.vector.tensor_copy(icr_s[:], icr_p[:])
            protT_s = wgt.tile([C, NC], F32, tag="protT")
            nc.vector.tensor_tensor(protT_s[:], sums_p, icr_s[:], mybir.AluOpType.mult)
            pwb_p = pps.tile([C, NC], F32, tag="pp")
            nc.tensor.matmul(pwb_p[:], wp_s[:], protT_s[:], start=True, stop=True)
            pwb_s = wgt.tile([C, NC], F32, tag="pwb")
            nc.vector.tensor_scalar(
                pwb_s[:], pwb_p[:], b1_s[:, 0:1], None, mybir.AluOpType.add
            )

            # ---- query prep for ALL chunks up front ----
            qT_all = wgt.tile([C, N_CHUNK * 128], F32R, tag="qT_all")
            cosT_all = wgt.tile([NC, N_CHUNK * 128], F32R, tag="cosT_all")
            invq_all = wgt.tile([128, N_CHUNK], F32, tag="invq_all")
            cosb_all = wgt.tile([128, N_CHUNK * NC], F32, tag="cosb_all")

            # qT transposes (independent of prototypes)
            def prep_qt_t(i):
                Q = qall[:, i * C : (i + 1) * C]
                qt_p = (pps if (i % 2 == 0 or i >= 8) else php).tile(
                    [C, 128], F32,
                    tag=("pp" if (i % 2 == 0 or i >= 8) else "ph0"), name="qt_p"
                )
                nc.tensor.transpose(qt_p[:], Q, id_s[:])
                nc.scalar.copy(qT_all[:, i * 128 : (i + 1) * 128], qt_p[:])

            def prep_qt_n(i):
                # 1/||q|| per row
                Q = qall[:, i * C : (i + 1) * C]
                sqd = smp.tile([128, C], F32, tag="sq")
                s2 = smp.tile([128, 1], F32, tag="s2")
                if i < 8:
                    nc.scalar.activation(
                        sqd[:], Q, mybir.ActivationFunctionType.Square,
                        accum_out=s2[:],
                    )
                else:
                    nc.vector.tensor_tensor(sqd[:], Q, Q, mybir.AluOpType.mult)
                    nc.vector.reduce_sum(s2[:], sqd[:], axis=mybir.AxisListType.X)
                nrmq = smp.tile([128, 1], F32, tag="nq")
                nc.scalar.activation(
                    nrmq[:], s2[:], mybir.ActivationFunctionType.Sqrt
                )
                nc.vector.reciprocal(invq_all[:, i : i + 1], nrmq[:])

            def prep_qt(i):
                prep_qt_t(i)
                prep_qt_n(i)

            for i in range(8):
                prep_qt(i)

            def prep_cos(i):
                sqn_p = pps.tile([128, NC], F32, tag="pp", name="sqn_p")
                nc.tensor.matmul(
                    sqn_p[:], qT_all[:, i * 128 : (i + 1) * 128], sumsT_sb[:],
                    start=True, stop=True,
                )
                # raw = (q . sums_n) * invq ; true cos = raw * ipn[n]
                coss = cosb_all[:, i * NC : (i + 1) * NC]
                nc.scalar.activation(
                    coss, sqn_p[:], mybir.ActivationFunctionType.Copy,
                    scale=invq_all[:, i : i + 1],
                )
                ct_p = pacc.tile([NC, 128], F32, tag="pp_sums", name="ct_p")
                nc.tensor.transpose(ct_p[:], coss, id_s[:])
                nc.vector.tensor_scalar(
                    cosT_all[:, i * 128 : (i + 1) * 128], ct_p[:],
                    ipnc_s[:, 0:1], None, mybir.AluOpType.mult,
                )

            # ---- main loop: tight matmul pipeline ----
            groups = []
            base = 0
            while base < N_CHUNK:
                w = min(4, N_CHUNK - base)
                groups.append((base, w))
                base += w

            for i in range(8):
                prep_cos(i)
            pending_qt = list(range(8, N_CHUNK))
            pending_cos = list(range(8, N_CHUNK))
            for gi, (gbase, gw) in enumerate(groups):
                W = gw * 128
                qTg = qT_all[:, gbase * 128 : gbase * 128 + W]
                cosTg = cosT_all[:, gbase * 128 : gbase * 128 + W]
                obs = [
                    obp.tile([128, 2 * NC * C], F32, tag=f"ob{j // 2}", name=f"ob{j // 2}")
                    for j in range(0, gw, 2)
                ]
                for ng in range(NC // 4):
                    if ng == 2:
                        for jp in range(0, gw, 2):
                            q0 = (gbase + jp) * 128
                            nw = min(2, gw - jp)
                            nc.sync.dma_start(
                                sim_d[q0 : q0 + nw * 128, 0:8, :].rearrange(
                                    "(a p) n c -> p a (n c)", p=128
                                ),
                                obs[jp // 2][:]
                                .rearrange("p (a c) -> p a c", c=NC * C)[
                                    :, :nw, : 8 * C
                                ],
                            )
                    hs = []
                    phs = []
                    for dn in range(4):
                        ph = php.tile([C, 512], F32, tag=f"ph{dn % 2}")
                        nc.tensor.matmul(
                            ph[:, :W], wq_s[:], qTg,
                            start=True, stop=False,
                        )
                        phs.append(ph)
                        if dn % 2 == 1:
                            for b in (0, 1):
                                n = ng * 4 + dn - 1 + b
                                nc.tensor.matmul(
                                    phs[dn - 1 + b][:, :W],
                                    wcm_s[:, n * C : (n + 1) * C],
                                    cosTg,
                                    start=False, stop=True,
                                )
                            for b in (0, 1):
                                n = ng * 4 + dn - 1 + b
                                h = hp.tile([C, 512], w2_dt, tag="h")
                                nc.scalar.activation(
                                    h[:, :W], phs[dn - 1 + b][:, :W],
                                    mybir.ActivationFunctionType.Relu,
                                    bias=pwb_s[:, n : n + 1],
                                )
                                hs.append(h)
                    for j in range(gw):
                        pg = psp.tile([128, 4 * C], F32, tag=f"psg{j % 3}")
                        for dn in range(4):
                            nc.tensor.matmul(
                                pg[:, dn * C : (dn + 1) * C],
                                hs[dn][:, j * 128 : (j + 1) * 128], w2_s[:],
                                start=True, stop=True,
                            )
                        nc.vector.tensor_tensor(
                            obs[j // 2][:, (j % 2) * NC * C + ng * 4 * C :
                                        (j % 2) * NC * C + (ng + 1) * 4 * C],
                            pg[:], b2r4_s[:], mybir.AluOpType.add,
                        )
                    if gi == 0:
                        for _ in range(6):
                            if pending_qt:
                                prep_qt(pending_qt.pop(0))
                    elif gi in (1, 2):
                        for _ in range(3):
                            if pending_cos:
                                prep_cos(pending_cos.pop(0))
                for jp in range(0, gw, 2):
                    q0 = (gbase + jp) * 128
                    nw = min(2, gw - jp)
                    nc.sync.dma_start(
                        sim_d[q0 : q0 + nw * 128, 8:NC, :].rearrange(
                            "(a p) n c -> p a (n c)", p=128
                        ),
                        obs[jp // 2][:]
                        .rearrange("p (a c) -> p a c", c=NC * C)[
                            :, :nw, 8 * C :
                        ],
                    )
                if gi == 5:
                    # cos output fixup (scale by per-class ipn) + store
                    nc.vector.tensor_tensor(
                        cosb_all[:].rearrange("p (a c) -> p a c", c=NC),
                        cosb_all[:].rearrange("p (a c) -> p a c", c=NC),
                        ipr_s[:].unsqueeze(1).broadcast_to(
                            [128, N_CHUNK, NC]
                        ),
                        mybir.AluOpType.mult,
                    )
                    nc.sync.dma_start(
                        cos_d[:, :].rearrange("(a p) c -> p a c", p=128),
                        cosb_all[:].rearrange("p (a c) -> p a c", c=NC),
                    )

    nc.finalize()
    return nc


def _prep_inputs(support_features, support_labels, query_features, W1, b1, W2, b2):
    """Host-side shard + constant prep. Returns per-core input maps."""
    sup = np.ascontiguousarray(np.asarray(support_features, np.float32))
    lab = np.asarray(support_labels).astype(np.float32).reshape(SUP_CHUNKS, 128)
    qf = np.ascontiguousarray(np.asarray(query_features, np.float32))
    W1 = np.asarray(W1, np.float32)
    b1 = np.asarray(b1, np.float32)
    W2 = np.asarray(W2, np.float32)
    b2 = np.asarray(b2, np.float32)

    wq = np.ascontiguousarray(W1[:C])
    wp = np.ascontiguousarray(W1[C : 2 * C])
    wc = W1[2 * C]
    wcm = np.zeros((NC, NC * C), np.float32)
    for n in range(NC):
        wcm[n, n * C : (n + 1) * C] = wc
    b1c = np.ascontiguousarray(b1.reshape(C, 1))
    w2 = np.ascontiguousarray(W2.astype(ml_dtypes.bfloat16 if FINAL_MM_BF16 else np.float32))
    b2r = np.ascontiguousarray(np.tile(b2.reshape(1, C), (128, 4)))
    ident = np.eye(128, dtype=np.float32)
    iota16 = np.tile(np.arange(NC, dtype=np.float32), (128, 1))
    onec = np.ones((128, 1), np.float32)
    oner = np.ones((1, 128), np.float32)

    in_maps = []
    for i in range(N_CORES):
        shard = np.zeros((QPAD, C), np.float32)
        shard[:QSHARD] = qf[i * QSHARD : (i + 1) * QSHARD]
        in_maps.append(
            {
                "q": shard, "sup": sup, "lab": lab, "wq": wq, "wp": wp,
                "wcm": wcm, "b1": b1c, "w2": w2, "b2r": b2r, "ident": ident,
                "iota16": iota16, "onec": onec, "oner": oner,
            }
        )
    return in_maps


def _run(inputs, trace=False):
    if "prog" not in _PROG_CACHE:
        _PROG_CACHE["prog"] = _build_program()
    nc = _PROG_CACHE["prog"]
    in_maps = _prep_inputs(**inputs)
    res = run_bass_kernel_spmd(nc, in_maps, list(range(N_CORES)), trace=trace)
    sim = np.concatenate([res.results[i]["sim"][:QSHARD] for i in range(N_CORES)], 0)
    cos = np.concatenate([res.results[i]["cosv"][:QSHARD] for i in range(N_CORES)], 0)
    return (sim, cos[:, :, None].astype(np.float32)), res


def kernel(**inputs):
    out, _ = _run(inputs, trace=False)
    return out
